# revision 17
# baseline (speedup 1.0000x reference)
"""MultiHeadAttention Trainium2 kernel.

Full inputs -> full output. Sharding: 8 cores = (batch b in 0..3) x (query
half in 0..1). Each core computes attention for its 1024 query rows of batch
b over all 2048 keys of batch b (K/V projections duplicated across the pair
of cores sharing a batch), then applies the output projection for its rows.
Outputs are disjoint row-slices of (B, S, D); host assembly is a pure concat.

All matmuls run in float32r (fp32 data at ~1 cycle/row, ~2^-12 rounding).

  phase A: transpose x row-blocks via PE-identity transposes, project
           Q^T [d,s], K^T [d,s], V [s,d] (V augmented with a ones column so
           the softmax denominator falls out of the ctx matmul); weight
           matrices stream per 128-row chunk; rank-1 matmuls add bq/bk.
  phase B: software-pipelined (head, query-block) iterations: logits^T
           [sk,sq] = K_h^T.T @ Q_h^T, P = exp(0.125*logits + mask*(-1e9))
           on the scalar engine, interleaved on the PE with the previous
           iteration's ctx matmuls; raw ctx/denominator rows go to DRAM.
  phase C: one batched reciprocal over all 32 denominator rows, PE
           broadcast of the recips, normalize raw ctx^T, then
           out = ctx @ wo + bo' where bo' = bo + bv @ wo (host-folded).
"""

import numpy as np

import concourse.bass as bass
import concourse.mybir as mybir
import concourse.tile as tile
from concourse import bacc
from concourse.bass_utils import run_bass_kernel_spmd

f32 = mybir.dt.float32
f32r = mybir.dt.float32r

B, S, D, H, DH = 4, 2048, 1024, 16, 64
SQ = S // 2          # query rows per core
N_CORES = 8
Exp = mybir.ActivationFunctionType.Exp

KC = D // 128        # 8 contraction chunks
SKC = S // 128       # 16 key chunks
NT = 2 * H           # 32 pipelined iterations (head, sqb)


def _build():
    nc = bacc.Bacc(None, target_bir_lowering=False)

    xq = nc.dram_tensor("xq", [D, SQ], f32r, kind="ExternalInput")   # query^T
    xk = nc.dram_tensor("xk", [D, S], f32r, kind="ExternalInput")    # key^T
    xv = nc.dram_tensor("xv", [D, S], f32r, kind="ExternalInput")    # value^T
    wq = nc.dram_tensor("wq", [D, D], f32r, kind="ExternalInput")
    wk = nc.dram_tensor("wk", [D, D], f32r, kind="ExternalInput")
    wv = nc.dram_tensor("wv", [D, D], f32r, kind="ExternalInput")
    wo = nc.dram_tensor("wo", [D, D], f32r, kind="ExternalInput")
    b3 = nc.dram_tensor("b3", [65, D], f32r, kind="ExternalInput")    # bq/bk/bo' at rows 0/32/64
    mb = nc.dram_tensor("mb", [128, SKC], f32, kind="ExternalInput")  # mask*-1e9 [p, chunk]
    one = nc.dram_tensor("one", [128, 512], f32r, kind="ExternalInput")
    sel = nc.dram_tensor("sel", [NT, 2, KC, 128], f32r, kind="ExternalInput")
    out = nc.dram_tensor("out", [SQ, D], f32, kind="ExternalOutput")

    with tile.TileContext(nc) as tc:
        _emit(nc, tc, xq, xk, xv, wq, wk, wv, wo, b3, mb, one, sel, out)
    nc.finalize()
    return nc


def _emit(nc, tc, xq, xk, xv, wq, wk, wv, wo, b3, mb, one, sel, out):
    from contextlib import ExitStack

    with ExitStack() as ctx:
        consts = ctx.enter_context(tc.tile_pool(name="consts", bufs=1))
        wpool = ctx.enter_context(tc.tile_pool(name="wpool", bufs=9))
        xtp = ctx.enter_context(tc.tile_pool(name="xtp", bufs=2))
        xtp2 = ctx.enter_context(tc.tile_pool(name="xtp2", bufs=2))
        qts = ctx.enter_context(tc.tile_pool(name="qts", bufs=2))
        kts = ctx.enter_context(tc.tile_pool(name="kts", bufs=2))
        vas = ctx.enter_context(tc.tile_pool(name="vas", bufs=2))
        ptp = ctx.enter_context(tc.tile_pool(name="ptp", bufs=10))
        stg = ctx.enter_context(tc.tile_pool(name="stg", bufs=2))
        cns2 = ctx.enter_context(tc.tile_pool(name="cns2", bufs=2))
        psA = ctx.enter_context(tc.tile_pool(name="psA", bufs=2, space="PSUM"))
        psX = ctx.enter_context(tc.tile_pool(name="psX", bufs=2, space="PSUM"))
        psC = ctx.enter_context(tc.tile_pool(name="psC", bufs=2, space="PSUM"))
        dram = ctx.enter_context(tc.tile_pool(name="dram", bufs=1, space="DRAM"))

        ktd = dram.tile([D, S], f32r)                 # K^T
        vad = dram.tile([SKC, 128, H, DH + 1], f32r)  # V augmented with ones col
        qtd = dram.tile([D, SQ], f32r)                # Q^T
        crd = dram.tile([D, SQ], f32r)                # raw (unnormalized) ctx^T
        dnd = dram.tile([NT, 512], f32r)              # denominator rows

        ones = consts.tile([128, 512], f32r)
        nc.sync.dma_start(ones, one[:])
        b3_sb = consts.tile([65, D], f32r)
        nc.sync.dma_start(b3_sb, b3[:])
        mb_sb = consts.tile([128, SKC], f32)
        nc.sync.dma_start(mb_sb, mb[:])
        sel_sb = consts.tile([NT, 2, KC, 128], f32r)
        nc.sync.dma_start(sel_sb, sel[:])

        def load_w(w_dram):
            chunks = []
            for kc in range(KC):
                wt = wpool.tile([128, D], f32r, tag="w", name="wt")
                nc.sync.dma_start(wt, w_dram[kc * 128:(kc + 1) * 128, :])
                chunks.append(wt)
            return chunks

        # ================= phase A =================
        def project_T(w_ch, brow, x_dram, blk, dst_dram):
            """One 512-row block of a transposed projection -> dst_dram."""
            xT = xtp.tile([128, KC, 512], f32r, tag="xT", name="xT")
            nc.sync.dma_start(xT, x_dram[:, blk * 512:(blk + 1) * 512]
                              .rearrange("(ko p) s -> p ko s", p=128))
            for dc2 in range(KC // 2):
                ps = psA.tile([128, 1024], f32, tag="psA", name="ps")
                for half in range(2):
                    dc = dc2 * 2 + half
                    ph = ps[:, half * 512:(half + 1) * 512]
                    for kc in range(KC):
                        nc.tensor.matmul(ph, lhsT=w_ch[kc][:, dc * 128:(dc + 1) * 128],
                                         rhs=xT[:, kc, :], start=(kc == 0), stop=False)
                    nc.tensor.matmul(ph, lhsT=b3_sb[brow:brow + 1, dc * 128:(dc + 1) * 128],
                                     rhs=ones[brow:brow + 1, 0:512], start=False, stop=True)
                st_t = stg.tile([128, 1024], f32r, tag="stg", name="st_t")
                nc.vector.tensor_copy(st_t, ps)
                for half in range(2):
                    dc = dc2 * 2 + half
                    nc.sync.dma_start(
                        dst_dram[dc * 128:(dc + 1) * 128, blk * 512:(blk + 1) * 512],
                        st_t[:, half * 512:(half + 1) * 512])

        wq_ch = load_w(wq)
        for sqb in range(2):
            project_T(wq_ch, 0, xq, sqb, qtd)

        wk_ch = load_w(wk)
        for skb in range(4):
            project_T(wk_ch, 32, xk, skb, ktd)

        wv_ch = load_w(wv)
        for sc in range(SKC):
            nc.sync.dma_start(vad[sc, :, :, DH], ones[:, 0:H])
        for sc in range(SKC):
            xvt = xtp2.tile([128, KC, 128], f32r, tag="xv", name="xvt")
            nc.sync.dma_start(xvt, xv[:, sc * 128:(sc + 1) * 128]
                              .rearrange("(ko p) s -> p ko s", p=128))
            ps = psA.tile([128, 1024], f32, tag="psA", name="ps")
            for dh2 in range(2):
                ph = ps[:, dh2 * 512:(dh2 + 1) * 512]
                for kc in range(KC):
                    nc.tensor.matmul(ph, lhsT=xvt[:, kc, :],
                                     rhs=wv_ch[kc][:, dh2 * 512:(dh2 + 1) * 512],
                                     start=(kc == 0), stop=(kc == KC - 1))
            st_t = stg.tile([128, 1024], f32r, tag="stg", name="st_t")
            nc.vector.tensor_copy(st_t, ps)
            nc.sync.dma_start(
                vad[sc, :, :, 0:DH],
                st_t.rearrange("p (h d) -> p h d", h=16),
            )

        # ================= phase B: software-pipelined attention =========
        state = {}

        def emit_logits_pair(t, skc2):
            st_ = state[t]
            psl = psA.tile([128, 1024], f32, tag="psA", name="psl")
            for half in range(2):
                skc = skc2 * 2 + half
                nc.tensor.matmul(psl[:, half * 512:(half + 1) * 512],
                                 lhsT=st_["kt"][:, skc * 128:(skc + 1) * 128],
                                 rhs=st_["qt"][:],
                                 start=True, stop=True)
            pt_t = ptp.tile([128, 2, 512], f32r, tag="pt", name="pt_t")
            nc.scalar.activation(
                pt_t.rearrange("p a b -> p (a b)"), psl, Exp,
                bias=mb_sb[:, skc2 * 2:skc2 * 2 + 1], scale=0.125)
            st_["pt"].append(pt_t)

        def emit_ctx_chunk(t, skc):
            st_ = state[t]
            if skc == 0:
                st_["psc"] = psC.tile([128, 512], f32, tag="psC", name="psc")
            nc.tensor.matmul(st_["psc"][0:DH + 1, :], lhsT=st_["va"][:, skc, :],
                             rhs=st_["pt"][skc // 2][:, skc % 2, :],
                             start=(skc == 0), stop=(skc == SKC - 1))

        def emit_store(t):
            st_ = state[t]
            h, sqb = st_["h"], st_["sqb"]
            cu = stg.tile([65, 512], f32r, tag="cu", name="cu")
            with nc.allow_low_precision(reason="raw ctx rounded to f32r"):
                nc.vector.tensor_copy(cu, st_["psc"][0:DH + 1, :])
            nc.sync.dma_start(crd[h * 64:(h + 1) * 64, sqb * 512:(sqb + 1) * 512],
                              cu[0:DH, :])
            nc.sync.dma_start(dnd[t:t + 1, :], cu[DH:DH + 1, :])
            del state[t]

        cur_kt = cur_va = None
        for t in range(NT):
            h, sqb = divmod(t, 2)
            base = (h % 2) * 64
            st_ = state[t] = {"h": h, "sqb": sqb, "base": base, "pt": []}
            if sqb == 0:
                cur_kt = kts.tile([64, S], f32r, tag="kt", name="kt")
                nc.sync.dma_start(cur_kt, ktd[h * 64:(h + 1) * 64, :])
                cur_va = vas.tile([128, SKC, DH + 1], f32r, tag="va", name="va")
                nc.sync.dma_start(cur_va, vad[:, :, h, :].rearrange("sc p c -> p sc c"))
            st_["kt"], st_["va"] = cur_kt, cur_va
            qt = qts.tile([64, 512], f32r, tag="qt", name="qt")
            nc.sync.dma_start(qt,
                              qtd[h * 64:(h + 1) * 64, sqb * 512:(sqb + 1) * 512])
            st_["qt"] = qt

            for skc2 in range(SKC // 2):
                emit_logits_pair(t, skc2)
                if t >= 1:
                    emit_ctx_chunk(t - 1, skc2 * 2)
                    emit_ctx_chunk(t - 1, skc2 * 2 + 1)
            if t >= 1:
                emit_store(t - 1)

        for skc in range(SKC):
            emit_ctx_chunk(NT - 1, skc)
        emit_store(NT - 1)

        # ================= phase C: normalize + output projection =========
        wo_ch = load_w(wo)
        den_sb = consts.tile([NT, 512], f32r)
        nc.sync.dma_start(den_sb, dnd[:])
        recf = consts.tile([NT, 512], f32)
        nc.vector.reciprocal(recf, den_sb)
        rec = consts.tile([NT, 512], f32r)
        with nc.allow_low_precision(reason="softmax recip rounded to f32r"):
            nc.vector.tensor_copy(rec, recf)

        for sqb in range(2):
            rb = xtp.tile([128, KC, 512], f32r, tag="xT", name="rb")
            for kc in range(KC):
                pb = psX.tile([128, 512], f32, tag="aux", name="pb")
                nc.tensor.matmul(pb, lhsT=sel_sb[:, sqb, kc, :], rhs=rec[:],
                                 start=True, stop=True)
                with nc.allow_low_precision(reason="recip bcast rounded to f32r"):
                    nc.vector.tensor_copy(rb[:, kc, :], pb)
            for st4 in range(4):
                st8 = sqb * 4 + st4
                cT = cns2.tile([128, KC, 128], f32r, tag="cT", name="cT")
                nc.sync.dma_start(cT, crd[:, st8 * 128:(st8 + 1) * 128]
                                  .rearrange("(ko p) s -> p ko s", p=128))
                with nc.allow_low_precision(reason="normalized ctx in f32r"):
                    nc.vector.tensor_mul(out=cT, in0=cT,
                                         in1=rb[:, :, st4 * 128:(st4 + 1) * 128])
                ps = psA.tile([128, 1024], f32, tag="psA", name="ps")
                for dh2 in range(2):
                    ph = ps[:, dh2 * 512:(dh2 + 1) * 512]
                    for kc in range(KC):
                        nc.tensor.matmul(ph, lhsT=cT[:, kc, :],
                                         rhs=wo_ch[kc][:, dh2 * 512:(dh2 + 1) * 512],
                                         start=(kc == 0), stop=False)
                    nc.tensor.matmul(ph, lhsT=ones[64:65, 0:128],
                                     rhs=b3_sb[64:65, dh2 * 512:(dh2 + 1) * 512],
                                     start=False, stop=True)
                st_t = stg.tile([128, 1024], f32, tag="ost", name="ost")
                nc.vector.tensor_copy(st_t, ps)
                nc.sync.dma_start(out[st8 * 128:(st8 + 1) * 128, :], st_t)


_NC_CACHE = None


def _selector():
    s = np.zeros((NT, 2, KC, 128), np.float32)
    for kc in range(KC):
        for p in range(128):
            h = 2 * kc + p // 64
            for sqb in range(2):
                s[2 * h + sqb, sqb, kc, p] = 1.0
    return s


def kernel(query, key, value, mask, wq, bq, wk, bk, wv, bv, wo, bo):
    global _NC_CACHE
    if _NC_CACHE is None:
        _NC_CACHE = _build()
    nc = _NC_CACHE

    query = np.asarray(query, dtype=np.float32)
    key = np.asarray(key, dtype=np.float32)
    value = np.asarray(value, dtype=np.float32)
    mask = np.asarray(mask, dtype=np.float32)
    kT = [np.ascontiguousarray(key[b].T) for b in range(B)]
    vT = [np.ascontiguousarray(value[b].T) for b in range(B)]
    wo_np = np.asarray(wo, np.float32)
    # fold the V bias through the output projection: (ctx + bv) @ wo + bo
    bo_eff = (np.asarray(bo, np.float64) +
              np.asarray(bv, np.float64) @ np.asarray(wo_np, np.float64)
              ).astype(np.float32)
    b3_host = np.zeros((65, D), np.float32)
    b3_host[0] = np.asarray(bq, np.float32)
    b3_host[32] = np.asarray(bk, np.float32)
    b3_host[64] = bo_eff

    shared = {
        "wq": np.asarray(wq, np.float32), "wk": np.asarray(wk, np.float32),
        "wv": np.asarray(wv, np.float32), "wo": wo_np,
        "b3": b3_host,
        "one": np.ones((128, 512), np.float32),
        "sel": _selector(),
    }
    in_maps = []
    for core in range(N_CORES):
        b, half = divmod(core, 2)
        mbc = np.ascontiguousarray(
            (mask[b, 0, 0] * np.float32(-1e9)).reshape(S // 128, 128).T)
        in_maps.append({
            "xq": np.ascontiguousarray(query[b, half * SQ:(half + 1) * SQ].T),
            "xk": kT[b], "xv": vT[b], "mb": mbc, **shared,
        })

    res = run_bass_kernel_spmd(nc, in_maps, core_ids=list(range(N_CORES)))
    full = np.empty((B, S, D), np.float32)
    for core in range(N_CORES):
        b, half = divmod(core, 2)
        full[b, half * SQ:(half + 1) * SQ] = res.results[core]["out"]
    return full
